# revision 1
# baseline (speedup 1.0000x reference)
"""Trainium2 Bass kernel for grouped per-block linear:
    y[b, g] = sum_d x[b, g*6+d] * W[g, d] + b[g]
x: [4194304, 60] f32 -> y: [4194304, 10] f32

Strategy (pure data parallel, 8 cores):
  - shard x by batch into 8 contiguous row blocks of 524288 rows
  - per core: tiles of [128 partitions, T=64 rows/partition], partition-major
    rows so every DMA is per-partition-contiguous in DRAM (15360 B lines).
  - Compute pipeline per tile (all engines stay far below the DMA roofline):
      Scalar (Act): convert x f32 -> fp16               (~1.9 us)
      DVE: p[0:6] = xh * Wh      (fp16, 2x_1p mode)     (~2.1 us)
      DVE: a = p[0:4] + p[4:8]   (fp16, 2x)             (~1.4 us)
      DVE: c = a[0:2] + a[2:4]   (fp16, 2x)             (~0.7 us)
      DVE: y = c[0] + c[1]       (f32 out)              (~0.7 us)
      store y via gpsimd-queue DMA
    The product tile p is [t, g, 8] with col 6 pre-filled with the bias and
    col 7 with 0 (filled once at startup; the mul only ever writes cols 0:6),
    so the add-tree folds the bias in for free.
  - fp16 intermediates halve DVE time (2x_1p needs all operands 2-byte +
    packed); rounding error ~2^-11 per stage, ~1e-3 max rel err overall.
  - Weights/bias consts are tiny ([128,60] + [128,80] fp16, ~36 KB total)
    and expanded on-chip via stride-0 broadcast views - DMA traffic is just
    x in (125.8 MB) + y out (21.0 MB) per core, the problem's I/O floor.
"""

import numpy as np

# ---------------- hardcoded problem constants ----------------
B_TOTAL = 4_194_304
N_CORES = 8
R = B_TOTAL // N_CORES  # 524288 rows per core
G = 10                  # groups
D = 6                   # group dim
DW = G * D              # 60 features per row
W8 = G * 8              # 80 = padded product-tile row width
P = 128                 # partitions
T = 64                  # rows per partition per tile
TILE_ROWS = P * T       # 8192 rows per tile
N_TILES = R // TILE_ROWS  # 64 iterations

_CACHE = {}


def _build_bass():
    import concourse.bacc as bacc
    import concourse.mybir as mybir
    import concourse.tile as tile

    f32 = mybir.dt.float32
    f16 = mybir.dt.float16
    nc = bacc.Bacc("TRN2", target_bir_lowering=False, debug=False)

    xs = nc.dram_tensor("xs", [R, DW], f32, kind="ExternalInput")
    wh = nc.dram_tensor("wh", [P, DW], f16, kind="ExternalInput")
    binit = nc.dram_tensor("binit", [P, W8], f16, kind="ExternalInput")
    ys = nc.dram_tensor("ys", [R, G], f32, kind="ExternalOutput")

    # Dense per-tile mapping: tile n covers TILE_ROWS consecutive rows,
    # partition p owns T consecutive rows -> every load tile is one
    # contiguous 1.97 MB DRAM region (HBM page locality; interleaved or
    # strided layouts measurably stretch load packets under contention).
    xs_r = xs[:, :].rearrange("(n p t) d -> n p (t d)", p=P, t=T)
    ys_r = ys[:, :].rearrange("(n p t) g -> n p (t g)", p=P, t=T)

    # Software-pipeline skew: the f32->fp16 convert (Scalar engine) for
    # tile i+SKEW is issued before the DVE ops of tile i, so the Tile
    # framework's conservative cross-engine waits reference DVE work SKEW
    # tiles back and the Scalar engine never serializes against the DVE.
    SKEW = 3

    with tile.TileContext(nc) as tc:
        with (
            tc.tile_pool(name="consts", bufs=1) as cpool,
            tc.tile_pool(name="xin", bufs=8) as xpool,
            tc.tile_pool(name="xh16", bufs=SKEW + 1) as hpool,
            tc.tile_pool(name="prods", bufs=1) as ppool,
            tc.tile_pool(name="lvla", bufs=2) as apool,
            tc.tile_pool(name="lvlb", bufs=2) as bpool,
            tc.tile_pool(name="yout", bufs=4) as ypool,
        ):
            # first x loads go ahead of the tiny const DMAs in the queue
            xt0 = xpool.tile([P, T * DW], f32, tag="x")
            nc.sync.dma_start(xt0, xs_r[0])
            xt1 = xpool.tile([P, T * DW], f32, tag="x")
            nc.sync.dma_start(xt1, xs_r[1])

            wt = cpool.tile([P, DW], f16, tag="wh")
            nc.sync.dma_start(wt, wh[:, :])
            # [P, 60] -> [P, T, G, D] with t-stride 0 (broadcast view)
            wt4 = wt.rearrange("p (o g d) -> p o g d", o=1, g=G, d=D)
            wt4 = wt4.broadcast_to((P, T, G, D))

            bi = cpool.tile([P, W8], f16, tag="binit")
            nc.sync.dma_start(bi, binit[:, :])
            bi3 = bi.rearrange("p (o w) -> p o w", o=1).broadcast_to((P, T, W8))

            # Two persistent product tiles [t, g, 8]; cols 6 (bias) / 7 (0)
            # written once here, the per-tile mul only writes cols 0:6.
            p8s = []
            for k in range(2):
                tk = ppool.tile([P, T * W8], f16, tag=f"p8_{k}")
                # init on the DVE so the Scalar queue starts with the
                # first x convert as soon as tile 0 lands
                nc.vector.tensor_copy(
                    tk.rearrange("p (t w) -> p t w", t=T), bi3
                )
                p8s.append(tk)

            xhs = {}
            for it in range(N_TILES + SKEW):
                if it < N_TILES:
                    if it == 0:
                        xt = xt0
                    elif it == 1:
                        xt = xt1
                    else:
                        xt = xpool.tile([P, T * DW], f32, tag="x")
                        nc.sync.dma_start(xt, xs_r[it])
                    xh = hpool.tile([P, T * DW], f16, tag="xh")
                    nc.scalar.copy(xh, xt)
                    xhs[it] = xh

                if it < SKEW:
                    continue
                i = it - SKEW
                xh4 = xhs.pop(i).rearrange(
                    "p (t g d) -> p t g d", t=T, g=G, d=D
                )

                p8 = p8s[i % 2]
                p84 = p8.rearrange("p (t g e) -> p t g e", t=T, g=G, e=8)
                nc.vector.tensor_tensor(
                    p84[:, :, :, 0:D], xh4, wt4, mybir.AluOpType.mult
                )

                at = apool.tile([P, T * G * 4], f16, tag="a")
                at4 = at.rearrange("p (t g e) -> p t g e", t=T, g=G, e=4)
                nc.vector.tensor_tensor(
                    at4, p84[:, :, :, 0:4], p84[:, :, :, 4:8],
                    mybir.AluOpType.add,
                )

                bt = bpool.tile([P, T * G * 2], f16, tag="b")
                bt4 = bt.rearrange("p (t g e) -> p t g e", t=T, g=G, e=2)
                nc.vector.tensor_tensor(
                    bt4, at4[:, :, :, 0:2], at4[:, :, :, 2:4],
                    mybir.AluOpType.add,
                )

                # final add + store trigger both on the otherwise-idle
                # GPSIMD: frees ~0.8 us/tile of DVE and keeps the store
                # dependent only on GPSIMD program order.  Stores stay
                # per-tile (2560 B lines): larger batched store bursts
                # measurably stretch concurrent load packets (HBM r/w
                # turnaround), costing far more than they save.
                yt = ypool.tile([P, T * G], f32, tag="y")
                yt4 = yt.rearrange("p (t g e) -> p t g e", t=T, g=G, e=1)
                nc.gpsimd.tensor_tensor(
                    yt4, bt4[:, :, :, 0:1], bt4[:, :, :, 1:2],
                    mybir.AluOpType.add,
                )
                nc.gpsimd.dma_start(ys_r[i], yt)

    nc.compile()
    return nc


def _get_bass():
    if "nc" not in _CACHE:
        _CACHE["nc"] = _build_bass()
    return _CACHE["nc"]


def _host_consts(W, b):
    # wh[p, g*6 + d] = W[g, d]  (fp16, broadcast over t on-chip)
    wflat = np.ascontiguousarray(W, dtype=np.float16).reshape(DW)
    wh = np.tile(wflat, (P, 1)).astype(np.float16)
    # binit[p, g*8 + j] = b[g] if j == 6 else 0
    brow = np.zeros((G, 8), dtype=np.float16)
    brow[:, 6] = np.asarray(b, dtype=np.float16)
    binit = np.tile(brow.reshape(W8), (P, 1)).astype(np.float16)
    return np.ascontiguousarray(wh), np.ascontiguousarray(binit)


def _run(x, W, b, **spmd_kwargs):
    from concourse import bass_utils

    x = np.ascontiguousarray(x, dtype=np.float32)
    assert x.shape == (B_TOTAL, DW), x.shape
    wh, binit = _host_consts(W, b)

    nc = _get_bass()
    in_maps = []
    for c in range(N_CORES):
        shard = x[c * R : (c + 1) * R]
        in_maps.append({"xs": shard, "wh": wh, "binit": binit})

    res = bass_utils.run_bass_kernel_spmd(
        nc, in_maps, core_ids=list(range(N_CORES)), **spmd_kwargs
    )
    y = np.concatenate([r["ys"] for r in res.results], axis=0)
    return y, res


def kernel(x, W, b):
    return _run(x, W, b)[0]

